# revision 1
# baseline (speedup 1.0000x reference)
"""LogEig kernel for Trainium2: batched matrix logarithm of SPD 64x64 matrices.

logm(X) via inverse scaling-and-squaring with Newton-Schulz iterations,
realized entirely with matmuls + elementwise ops (no eigendecomposition):

  X' = X/c; three "E-form" NS-sqrt stages (E = I - z*y residual recurrence,
  numerically stable in fp32), one (Y, Z/2)-form final stage, then
  W = (S - S^-1)/2 = sinh(log S) and a degree-9 odd polynomial for asinh:
  logm(X) = P(W) + ln(c) I   (2^K folded into P).

Data layout: matrices processed in pair-stacked blocks: 2 matrices per
128 partitions (even matrix rows 0:64, odd matrix rows 64:128), G pairs per
block. Stationaries are block-diagonal [128,128] (zeros kept persistent in
dedicated tiles), moving operands are the pair-stacked tiles.

Self-contained: coefficients are inlined; host code shards over 8 cores.
"""
import numpy as np
from contextlib import ExitStack

from concourse import bass, tile
from concourse.bass import mybir
from concourse.bass_utils import run_bass_kernel_spmd

F32 = mybir.dt.float32
ALU = mybir.AluOpType

# ---- chain coefficients (designed offline; see gen_coeffs) ----
C_NORM = 7.0
LN_C = 1.9459101490553132
STAGE_ITERS = [8, 5, 4, 3]
AB = [
    (3.7542098559612636, -3.9283413904351194),
    (2.5530521787582194, -1.2404429025056762),
    (2.457078973800643, -1.1397346701527205),
    (2.1926406947022983, -0.9262702911062604),
    (1.7559003594756186, -0.6442488802289593),
    (1.5258024236104812, -0.5140913265998878),
    (1.5003437888937057, -0.500342370657249),
    (1.4989979345962765, -0.4989986038705908),
    (3.40778435255814, -3.1321516827360614),
    (2.034227922250582, -0.8177603407135465),
    (1.6296142922792152, -0.5704059772933627),
    (1.5057102487512888, -0.5026917588204257),
    (1.5003625941833543, -0.5003611847474739),
    (2.580963980830702, -1.824796692998573),
    (1.5692437161914854, -0.5376121481144274),
    (1.5032635884449788, -0.5023978154068648),
    (1.500404041336444, -0.5004026529624388),
    (1.9971494210242315, -1.0599713802766355),
    (1.504870004021479, -0.5026229555260626),
    (1.5005566192233029, -0.5005553281253057),
]
POLY = [
    16.000064987184754,
    -2.6724424886480778,
    1.272392023482041,
    -0.9878048401218855,
    0.7087224370083787,
]

def _legalize_waits(nc, max_waits=1):
    """walrus on this toolchain accepts only ~1 sync-wait per instruction;
    split excess waits onto preceding same-engine NoOps (in-order engines,
    so this is semantics-preserving)."""
    for f in nc.m.functions:
        for bb in f.blocks:
            insts = bb.instructions
            i = 0
            while i < len(insts):
                ins = insts[i]
                si = getattr(ins, "sync_info", None)
                if si is None or not si.on_wait or len(si.on_wait) <= max_waits:
                    i += 1
                    continue
                waits = list(si.on_wait)
                for w in waits[:-max_waits]:
                    nop = mybir.InstNoOp(
                        name=nc.get_next_instruction_name(), ins=[], outs=[],
                        engine=ins.engine,
                        sync_info=mybir.SyncInfo(on_wait=[w], on_update=[]),
                        bass_nofuse=True)
                    insts.insert(i, nop)
                    i += 1
                si.on_wait = waits[-max_waits:]
                ins.sync_info = si
                i += 1


B_TOTAL, N = 8192, 64
N_CORES = 8
BPC = B_TOTAL // N_CORES        # 1024 matrices per core
G = 8                           # pairs per block
MPB = 2 * G                     # matrices per block
NB = BPC // MPB                 # blocks per core

# const-bank layout: [one, c0 per E-iter(17), a per yz-iter(3), p3,p2,p1,p0, lnc]
N_EITER = sum(STAGE_ITERS[:-1])
N_ZITER = STAGE_ITERS[-1]
NCONST = 1 + N_EITER + N_ZITER + 4 + 1


def _host_consts():
    ident = np.zeros((128, 64), dtype=np.float32)
    for p in range(128):
        ident[p, p % 64] = 1.0
    bank = np.zeros((128, NCONST, 64), dtype=np.float32)
    bank[:, 0, :] = ident
    j = 1
    for (a, b) in AB[:N_EITER]:
        s = a + b
        bank[:, j, :] = np.float32(1.0 - s * s) * ident
        j += 1
    for (a, b) in AB[N_EITER:]:
        bank[:, j, :] = np.float32(a) * ident
        j += 1
    for p in (POLY[3], POLY[2], POLY[1], POLY[0]):
        bank[:, j, :] = np.float32(p) * ident
        j += 1
    bank[:, j, :] = np.float32(LN_C) * ident
    return bank


def to_dev_layout(x):
    """x [BPC,64,64] -> H [128, NB, G*64] pair-stacked."""
    xb = x.reshape(NB, G, 2, 64, 64)
    H = xb.transpose(2, 3, 0, 1, 4)
    return np.ascontiguousarray(H.reshape(128, NB, G * 64))


def from_dev_layout(H):
    xb = H.reshape(2, 64, NB, G, 64).transpose(2, 3, 0, 1, 4)
    return np.ascontiguousarray(xb.reshape(BPC, 64, 64))


def build_nc(nb=NB, use_for_i=False, reps=1, trip_override=None):
    nc = bass.Bass("TRN2")
    x_in = nc.declare_dram_parameter("x", [128, nb, G, 64], F32, isOutput=False)
    k_in = nc.declare_dram_parameter("konst", [128, NCONST, 64], F32, isOutput=False)
    y_out = nc.declare_dram_parameter("y", [128, nb, G, 64], F32, isOutput=True)

    with tile.TileContext(nc) as tc, ExitStack() as ctx:
        cpool = ctx.enter_context(tc.tile_pool(name="consts", bufs=1))
        bdpool = ctx.enter_context(tc.tile_pool(name="bd", bufs=1))
        eypool = ctx.enter_context(tc.tile_pool(name="ey", bufs=2))
        tpool = ctx.enter_context(tc.tile_pool(name="tmp", bufs=2))
        iopool = ctx.enter_context(tc.tile_pool(name="io", bufs=3))
        papool = ctx.enter_context(tc.tile_pool(name="psa", bufs=2, space="PSUM"))
        pbpool = ctx.enter_context(tc.tile_pool(name="psb", bufs=2, space="PSUM"))

        konst = cpool.tile([128, NCONST, 64], F32)
        nc.sync.dma_start(out=konst[:], in_=k_in[:])

        def kslice(idx):
            return konst[:, idx : idx + 1, :].broadcast_to([128, G, 64])

        # two persistent block-diag stationary tiles (off-diag quadrants stay 0)
        bd0 = bdpool.tile([128, G, 128], F32, tag="bd0")
        bd1 = bdpool.tile([128, G, 128], F32, tag="bd1")
        nc.vector.memset(bd0[:], 0.0)
        nc.vector.memset(bd1[:], 0.0)
        bds = [bd0, bd1]
        bd_i = [0]

        def next_bd():
            t = bds[bd_i[0] % 2]
            bd_i[0] += 1
            return t

        def mirror_to_bd(src64, bd, eng):
            """src64: [128, G, 64] stacked; write halves into bd quadrants."""
            eng.copy(bd[0:64, :, 0:64], src64[0:64])
            eng.copy(bd[64:128, :, 64:128], src64[64:128])

        def body(blk):
            xt = iopool.tile([128, G, 64], F32, tag="xin")
            nc.sync.dma_start(out=xt[:], in_=x_in[:, blk])

            # ---- stage 0 init: Y = X/c, E = I - X/c ----
            ey = eypool.tile([128, G, 128], F32, tag="ey")
            nc.vector.tensor_scalar_mul(ey[:, :, 64:128], xt[:], 1.0 / C_NORM)
            nc.vector.scalar_tensor_tensor(
                ey[:, :, 0:64], xt[:], -1.0 / C_NORM, kslice(0),
                op0=ALU.mult, op1=ALU.add)
            ebd = next_bd()
            mirror_to_bd(ey[:, :, 0:64], ebd, nc.scalar)

            it = 0
            for s_idx, n_it in enumerate(STAGE_ITERS[:-1]):
                if s_idx > 0:
                    # stage re-init: E = I - Y (Y half stays)
                    ey2 = eypool.tile([128, G, 128], F32, tag="ey")
                    nc.vector.tensor_copy(ey2[:, :, 64:128], ey[:, :, 64:128])
                    nc.vector.scalar_tensor_tensor(
                        ey2[:, :, 0:64], ey[:, :, 64:128], -1.0, kslice(0),
                        op0=ALU.mult, op1=ALU.add)
                    ey = ey2
                    ebd = next_bd()
                    mirror_to_bd(ey[:, :, 0:64], ebd, nc.scalar)
                for k in range(n_it):
                    a, b = AB[it]
                    sv, q = a + b, -b
                    c0 = 1.0 - sv * sv          # lives in konst slice 1+it
                    c1 = sv * sv - 2.0 * sv * q
                    c2 = 2.0 * sv * q - q * q
                    c3 = q * q
                    psa = papool.tile([128, G, 128], F32, tag="psa")
                    for g in range(G):
                        nc.tensor.matmul(
                            psa[:, g, :], ebd[:, g, :], ey[:, g, :],
                            start=True, stop=True)
                    usb = tpool.tile([128, G, 64], F32, tag="usb")
                    nc.scalar.mul(usb[:], psa[:, :, 0:64], c3)     # c3*E^2
                    yq = tpool.tile([128, G, 64], F32, tag="yq")
                    nc.scalar.mul(yq[:], psa[:, :, 64:128], q)     # q*E@Y
                    psb = pbpool.tile([128, G, 64], F32, tag="psb")
                    for g in range(G):
                        nc.tensor.matmul(
                            psb[:, g, :], ebd[:, g, :], usb[:, g, :],
                            start=True, stop=True)                 # c3*E^3
                    ey2 = eypool.tile([128, G, 128], F32, tag="ey")
                    # Y' = s*Y + q*EY
                    nc.vector.scalar_tensor_tensor(
                        ey2[:, :, 64:128], ey[:, :, 64:128], float(sv), yq[:],
                        op0=ALU.mult, op1=ALU.add)
                    # E' = c0*I + c1*E + (c2/c3)*(c3 E^2) + (c3 E^3)
                    t1 = tpool.tile([128, G, 64], F32, tag="t1")
                    nc.vector.scalar_tensor_tensor(
                        t1[:], usb[:], c2 / c3, kslice(1 + it),
                        op0=ALU.mult, op1=ALU.add)
                    t2 = tpool.tile([128, G, 64], F32, tag="t2")
                    nc.vector.scalar_tensor_tensor(
                        t2[:], ey[:, :, 0:64], float(c1), t1[:],
                        op0=ALU.mult, op1=ALU.add)
                    nc.vector.tensor_add(ey2[:, :, 0:64], psb[:], t2[:])
                    ey = ey2
                    if not (k == n_it - 1):
                        ebd = next_bd()
                        mirror_to_bd(ey[:, :, 0:64], ebd, nc.scalar)
                    it += 1

            # ---- final stage: (Y, Zh=Z/2) form ----
            yz = None
            zbd = None
            for k in range(STAGE_ITERS[-1]):
                a, b = AB[it]
                aslice = kslice(1 + N_EITER + k)
                if k == 0:
                    vbd = next_bd()
                    # Vh = a*I + b*Y  (write halves directly into bd quadrants)
                    nc.vector.scalar_tensor_tensor(
                        vbd[0:64, :, 0:64], ey[0:64, :, 64:128], b,
                        aslice[0:64], op0=ALU.mult, op1=ALU.add)
                    nc.vector.scalar_tensor_tensor(
                        vbd[64:128, :, 64:128], ey[64:128, :, 64:128], b,
                        aslice[64:128], op0=ALU.mult, op1=ALU.add)
                    psb = pbpool.tile([128, G, 64], F32, tag="psb")
                    for g in range(G):
                        nc.tensor.matmul(
                            psb[:, g, :], vbd[:, g, :], ey[:, g, 64:128],
                            start=True, stop=True)                 # Y' = Vh@Y
                    yz = eypool.tile([128, G, 128], F32, tag="ey")
                    nc.scalar.copy(yz[:, :, 0:64], psb[:])
                    # Zh = 0.5 * Vh  (from bd quadrants, per half)
                    nc.vector.tensor_scalar_mul(
                        yz[0:64, :, 64:128], vbd[0:64, :, 0:64], 0.5)
                    nc.vector.tensor_scalar_mul(
                        yz[64:128, :, 64:128], vbd[64:128, :, 64:128], 0.5)
                else:
                    zbd = next_bd()
                    mirror_to_bd(yz[:, :, 64:128], zbd, nc.scalar)
                    psb = pbpool.tile([128, G, 64], F32, tag="psb")
                    for g in range(G):
                        nc.tensor.matmul(
                            psb[:, g, :], zbd[:, g, :], yz[:, g, 0:64],
                            start=True, stop=True)                 # M = Zh@Y
                    vbd = next_bd()
                    nc.vector.scalar_tensor_tensor(
                        vbd[0:64, :, 0:64], psb[0:64], 2.0 * b,
                        aslice[0:64], op0=ALU.mult, op1=ALU.add)
                    nc.vector.scalar_tensor_tensor(
                        vbd[64:128, :, 64:128], psb[64:128], 2.0 * b,
                        aslice[64:128], op0=ALU.mult, op1=ALU.add)
                    psa = papool.tile([128, G, 128], F32, tag="psa")
                    for g in range(G):
                        nc.tensor.matmul(
                            psa[:, g, :], vbd[:, g, :], yz[:, g, :],
                            start=True, stop=True)                 # [Y'|Zh']
                    yz2 = eypool.tile([128, G, 128], F32, tag="ey")
                    nc.scalar.copy(yz2[:], psa[:])
                    yz = yz2
                it += 1

            # ---- W = 0.5*Y - Zh ; U = W@W ; odd poly ----
            wst = tpool.tile([128, G, 64], F32, tag="wst")
            nc.vector.scalar_tensor_tensor(
                wst[:], yz[:, :, 0:64], 0.5, yz[:, :, 64:128],
                op0=ALU.mult, op1=ALU.subtract)
            wbd = next_bd()
            mirror_to_bd(wst[:], wbd, nc.scalar)
            psb = pbpool.tile([128, G, 64], F32, tag="psb")
            for g in range(G):
                nc.tensor.matmul(psb[:, g, :], wbd[:, g, :], wst[:, g, :],
                                 start=True, stop=True)            # U = W@W
            usb = tpool.tile([128, G, 64], F32, tag="usb")
            nc.scalar.copy(usb[:], psb[:])
            ubd = next_bd()
            mirror_to_bd(usb[:], ubd, nc.scalar)
            tacc = tpool.tile([128, G, 64], F32, tag="tacc")
            nc.vector.scalar_tensor_tensor(
                tacc[:], usb[:], POLY[4], kslice(1 + N_EITER + N_ZITER),
                op0=ALU.mult, op1=ALU.add)                         # p4*U + p3*I
            for j in (2, 1, 0):
                psb = pbpool.tile([128, G, 64], F32, tag="psb")
                for g in range(G):
                    nc.tensor.matmul(psb[:, g, :], ubd[:, g, :], tacc[:, g, :],
                                     start=True, stop=True)        # U@T
                tacc2 = tpool.tile([128, G, 64], F32, tag="tacc")
                nc.vector.scalar_tensor_tensor(
                    tacc2[:], psb[:], 1.0, kslice(1 + N_EITER + N_ZITER + (3 - j)),
                    op0=ALU.mult, op1=ALU.add)
                tacc = tacc2
            psb = pbpool.tile([128, G, 64], F32, tag="psb")
            for g in range(G):
                nc.tensor.matmul(psb[:, g, :], wbd[:, g, :], tacc[:, g, :],
                                 start=True, stop=True)            # W @ P'(U)
            out_t = iopool.tile([128, G, 64], F32, tag="out")
            nc.vector.scalar_tensor_tensor(
                out_t[:], psb[:], 1.0, kslice(NCONST - 1),
                op0=ALU.mult, op1=ALU.add)                         # + ln(c) I
            nc.sync.dma_start(out=y_out[:, blk], in_=out_t[:])

        if use_for_i:
            if reps > 1:
                with tc.For_i(0, reps, 1) as i:
                    for blk in range(nb):
                        body(blk)
            else:
                with tc.For_i(0, trip_override if trip_override else nb, 1) as i:
                    body(i)
        else:
            for _rep in range(reps):
                for blk in range(nb):
                    body(blk)

    _legalize_waits(nc)
    return nc


_NC_CACHE = {}


def kernel(x: np.ndarray) -> np.ndarray:
    assert x.shape == (B_TOTAL, N, N)
    key = "full"
    if key not in _NC_CACHE:
        _NC_CACHE[key] = build_nc(use_for_i=True)
    nc = _NC_CACHE[key]
    kbank = _host_consts()
    in_maps = []
    for c in range(N_CORES):
        xc = np.ascontiguousarray(x[c * BPC : (c + 1) * BPC]).astype(np.float32)
        in_maps.append({"x": to_dev_layout(xc), "konst": kbank})
    res = run_bass_kernel_spmd(nc, in_maps, list(range(N_CORES)))
    out = np.empty((B_TOTAL, N, N), dtype=np.float32)
    for c in range(N_CORES):
        out[c * BPC : (c + 1) * BPC] = from_dev_layout(res.results[c]["y"])
    return out



# revision 3
# speedup vs baseline: 2.5031x; 2.5031x over previous
"""LogEig kernel for Trainium2: batched matrix logarithm of SPD 64x64 matrices.

logm(X) via inverse scaling-and-squaring with Newton-Schulz iterations,
realized entirely with matmuls + elementwise ops (no eigendecomposition):

  X' = X/c; three "E-form" NS-sqrt stages (E = I - z*y residual recurrence,
  numerically stable in fp32), one (Y, Z/2)-form final stage, then
  W = (S - S^-1)/2 = sinh(log S) and a degree-9 odd polynomial for asinh:
  logm(X) = P(W) + ln(c) I   (2^K folded into P).

Host<->device wire format is minimized (the axon tunnel is ~60-80 MB/s and
dominates wall clock): input ships as fp16 + int8 residual (3 B/elem,
~15 effective mantissa bits reconstructed on device), output ships as fp16.
The pair-stacked on-chip layout (2 matrices per 128 partitions) is produced
by the DMA access pattern directly from the natural [B,64,64] layout, so the
host does no data movement beyond dtype splits.

The jax/PJRT execution path (shard_map over 8 cores wrapping the bass_exec
custom call) is built ONCE and cached; konst bank and the dummy output
operand stay device-resident across calls.
"""
import numpy as np
from contextlib import ExitStack

from concourse import bass, tile
from concourse.bass import mybir
from concourse import bass2jax as b2j

F32 = mybir.dt.float32
F16 = mybir.dt.float16
I8 = mybir.dt.int8
ALU = mybir.AluOpType

# ---- chain coefficients (designed offline) ----
C_NORM = 7.0
LN_C = 1.9459101490553132
STAGE_ITERS = [8, 5, 4, 3]
AB = [
    (3.7542098559612636, -3.9283413904351194),
    (2.5530521787582194, -1.2404429025056762),
    (2.457078973800643, -1.1397346701527205),
    (2.1926406947022983, -0.9262702911062604),
    (1.7559003594756186, -0.6442488802289593),
    (1.5258024236104812, -0.5140913265998878),
    (1.5003437888937057, -0.500342370657249),
    (1.4989979345962765, -0.4989986038705908),
    (3.40778435255814, -3.1321516827360614),
    (2.034227922250582, -0.8177603407135465),
    (1.6296142922792152, -0.5704059772933627),
    (1.5057102487512888, -0.5026917588204257),
    (1.5003625941833543, -0.5003611847474739),
    (2.580963980830702, -1.824796692998573),
    (1.5692437161914854, -0.5376121481144274),
    (1.5032635884449788, -0.5023978154068648),
    (1.500404041336444, -0.5004026529624388),
    (1.9971494210242315, -1.0599713802766355),
    (1.504870004021479, -0.5026229555260626),
    (1.5005566192233029, -0.5005553281253057),
]
POLY = [
    16.000064987184754,
    -2.6724424886480778,
    1.272392023482041,
    -0.9878048401218855,
    0.7087224370083787,
]


def _legalize_waits(nc, max_waits=1):
    """walrus on this toolchain accepts only ~1 sync-wait per instruction;
    split excess waits onto preceding same-engine NoOps (in-order engines,
    so this is semantics-preserving)."""
    for f in nc.m.functions:
        for bb in f.blocks:
            insts = bb.instructions
            i = 0
            while i < len(insts):
                ins = insts[i]
                si = getattr(ins, "sync_info", None)
                if si is None or not si.on_wait or len(si.on_wait) <= max_waits:
                    i += 1
                    continue
                waits = list(si.on_wait)
                for w in waits[:-max_waits]:
                    nop = mybir.InstNoOp(
                        name=nc.get_next_instruction_name(), ins=[], outs=[],
                        engine=ins.engine,
                        sync_info=mybir.SyncInfo(on_wait=[w], on_update=[]),
                        bass_nofuse=True)
                    insts.insert(i, nop)
                    i += 1
                si.on_wait = waits[-max_waits:]
                ins.sync_info = si
                i += 1


B_TOTAL, N = 8192, 64
N_CORES = 8
BPC = B_TOTAL // N_CORES        # 1024 matrices per core
G = 8                           # pairs per block
MPB = 2 * G                     # matrices per block
NB = BPC // MPB                 # blocks per core

RES_SCALE = 16384.0             # int8 residual quantization scale (2^14)

# const-bank layout: [one, c0 per E-iter(17), a per yz-iter(3), p3,p2,p1,p0, lnc]
N_EITER = sum(STAGE_ITERS[:-1])
N_ZITER = STAGE_ITERS[-1]
NCONST = 1 + N_EITER + N_ZITER + 4 + 1


def _host_consts():
    ident = np.zeros((128, 64), dtype=np.float32)
    for p in range(128):
        ident[p, p % 64] = 1.0
    bank = np.zeros((128, NCONST, 64), dtype=np.float32)
    bank[:, 0, :] = ident
    j = 1
    for (a, b) in AB[:N_EITER]:
        s = a + b
        bank[:, j, :] = np.float32(1.0 - s * s) * ident
        j += 1
    for (a, b) in AB[N_EITER:]:
        bank[:, j, :] = np.float32(a) * ident
        j += 1
    for p in (POLY[3], POLY[2], POLY[1], POLY[0]):
        bank[:, j, :] = np.float32(p) * ident
        j += 1
    bank[:, j, :] = np.float32(LN_C) * ident
    return bank


def build_nc(nb=NB):
    nc = bass.Bass("TRN2")
    xh_in = nc.declare_dram_parameter("xh", [nb, G, 2, 64, 64], F16, isOutput=False)
    xl_in = nc.declare_dram_parameter("xl", [nb, G, 2, 64, 64], I8, isOutput=False)
    k_in = nc.declare_dram_parameter("konst", [128, NCONST, 64], F32, isOutput=False)
    y_out = nc.declare_dram_parameter("y", [nb, G, 2, 64, 64], F16, isOutput=True)

    with tile.TileContext(nc) as tc, ExitStack() as ctx:
        cpool = ctx.enter_context(tc.tile_pool(name="consts", bufs=1))
        bdpool = ctx.enter_context(tc.tile_pool(name="bd", bufs=1))
        eypool = ctx.enter_context(tc.tile_pool(name="ey", bufs=2))
        tpool = ctx.enter_context(tc.tile_pool(name="tmp", bufs=2))
        iopool = ctx.enter_context(tc.tile_pool(name="io", bufs=3))
        papool = ctx.enter_context(tc.tile_pool(name="psa", bufs=2, space="PSUM"))
        pbpool = ctx.enter_context(tc.tile_pool(name="psb", bufs=2, space="PSUM"))

        konst = cpool.tile([128, NCONST, 64], F32)
        nc.sync.dma_start(out=konst[:], in_=k_in[:])

        def kslice(idx):
            return konst[:, idx : idx + 1, :].broadcast_to([128, G, 64])

        # two persistent block-diag stationary tiles (off-diag quadrants stay 0)
        bd0 = bdpool.tile([128, G, 128], F32, tag="bd0")
        bd1 = bdpool.tile([128, G, 128], F32, tag="bd1")
        nc.vector.memset(bd0[:], 0.0)
        nc.vector.memset(bd1[:], 0.0)
        bds = [bd0, bd1]
        bd_i = [0]

        def next_bd():
            t = bds[bd_i[0] % 2]
            bd_i[0] += 1
            return t

        def mirror_to_bd(src64, bd, eng):
            """src64: [128, G, 64] stacked; write halves into bd quadrants."""
            eng.copy(bd[0:64, :, 0:64], src64[0:64])
            eng.copy(bd[64:128, :, 64:128], src64[64:128])

        def body(blk):
            # natural-layout DRAM view (g, s, r, c) -> iteration (s, r, g, c)
            # matches the pair-stacked SBUF tile [(s r)=128, G, 64] in
            # flattened element order, so the DMA does the relayout.
            ht = iopool.tile([128, G, 64], F16, tag="xh")
            nc.sync.dma_start(out=ht[:], in_=xh_in[blk].transpose([1, 2, 0, 3]))
            lt = iopool.tile([128, G, 64], I8, tag="xl")
            nc.sync.dma_start(out=lt[:], in_=xl_in[blk].transpose([1, 2, 0, 3]))
            xt = iopool.tile([128, G, 64], F32, tag="xin")
            nc.vector.scalar_tensor_tensor(
                xt[:], lt[:], 1.0 / RES_SCALE, ht[:],
                op0=ALU.mult, op1=ALU.add)

            # ---- stage 0 init: Y = X/c, E = I - X/c ----
            ey = eypool.tile([128, G, 128], F32, tag="ey")
            nc.vector.tensor_scalar_mul(ey[:, :, 64:128], xt[:], 1.0 / C_NORM)
            nc.vector.scalar_tensor_tensor(
                ey[:, :, 0:64], xt[:], -1.0 / C_NORM, kslice(0),
                op0=ALU.mult, op1=ALU.add)
            ebd = next_bd()
            mirror_to_bd(ey[:, :, 0:64], ebd, nc.scalar)

            it = 0
            for s_idx, n_it in enumerate(STAGE_ITERS[:-1]):
                if s_idx > 0:
                    # stage re-init: E = I - Y (Y half stays)
                    ey2 = eypool.tile([128, G, 128], F32, tag="ey")
                    nc.vector.tensor_copy(ey2[:, :, 64:128], ey[:, :, 64:128])
                    nc.vector.scalar_tensor_tensor(
                        ey2[:, :, 0:64], ey[:, :, 64:128], -1.0, kslice(0),
                        op0=ALU.mult, op1=ALU.add)
                    ey = ey2
                    ebd = next_bd()
                    mirror_to_bd(ey[:, :, 0:64], ebd, nc.scalar)
                for k in range(n_it):
                    a, b = AB[it]
                    sv, q = a + b, -b
                    c0 = 1.0 - sv * sv          # lives in konst slice 1+it
                    c1 = sv * sv - 2.0 * sv * q
                    c2 = 2.0 * sv * q - q * q
                    c3 = q * q
                    psa = papool.tile([128, G, 128], F32, tag="psa")
                    for g in range(G):
                        nc.tensor.matmul(
                            psa[:, g, :], ebd[:, g, :], ey[:, g, :],
                            start=True, stop=True)
                    usb = tpool.tile([128, G, 64], F32, tag="usb")
                    nc.scalar.mul(usb[:], psa[:, :, 0:64], c3)     # c3*E^2
                    yq = tpool.tile([128, G, 64], F32, tag="yq")
                    nc.scalar.mul(yq[:], psa[:, :, 64:128], q)     # q*E@Y
                    psb = pbpool.tile([128, G, 64], F32, tag="psb")
                    for g in range(G):
                        nc.tensor.matmul(
                            psb[:, g, :], ebd[:, g, :], usb[:, g, :],
                            start=True, stop=True)                 # c3*E^3
                    ey2 = eypool.tile([128, G, 128], F32, tag="ey")
                    # Y' = s*Y + q*EY
                    nc.vector.scalar_tensor_tensor(
                        ey2[:, :, 64:128], ey[:, :, 64:128], float(sv), yq[:],
                        op0=ALU.mult, op1=ALU.add)
                    # E' = c0*I + c1*E + (c2/c3)*(c3 E^2) + (c3 E^3)
                    t1 = tpool.tile([128, G, 64], F32, tag="t1")
                    nc.vector.scalar_tensor_tensor(
                        t1[:], usb[:], c2 / c3, kslice(1 + it),
                        op0=ALU.mult, op1=ALU.add)
                    t2 = tpool.tile([128, G, 64], F32, tag="t2")
                    nc.vector.scalar_tensor_tensor(
                        t2[:], ey[:, :, 0:64], float(c1), t1[:],
                        op0=ALU.mult, op1=ALU.add)
                    nc.vector.tensor_add(ey2[:, :, 0:64], psb[:], t2[:])
                    ey = ey2
                    if not (k == n_it - 1):
                        ebd = next_bd()
                        mirror_to_bd(ey[:, :, 0:64], ebd, nc.scalar)
                    it += 1

            # ---- final stage: (Y, Zh=Z/2) form ----
            yz = None
            zbd = None
            for k in range(STAGE_ITERS[-1]):
                a, b = AB[it]
                aslice = kslice(1 + N_EITER + k)
                if k == 0:
                    vbd = next_bd()
                    # Vh = a*I + b*Y  (write halves directly into bd quadrants)
                    nc.vector.scalar_tensor_tensor(
                        vbd[0:64, :, 0:64], ey[0:64, :, 64:128], b,
                        aslice[0:64], op0=ALU.mult, op1=ALU.add)
                    nc.vector.scalar_tensor_tensor(
                        vbd[64:128, :, 64:128], ey[64:128, :, 64:128], b,
                        aslice[64:128], op0=ALU.mult, op1=ALU.add)
                    psb = pbpool.tile([128, G, 64], F32, tag="psb")
                    for g in range(G):
                        nc.tensor.matmul(
                            psb[:, g, :], vbd[:, g, :], ey[:, g, 64:128],
                            start=True, stop=True)                 # Y' = Vh@Y
                    yz = eypool.tile([128, G, 128], F32, tag="ey")
                    nc.scalar.copy(yz[:, :, 0:64], psb[:])
                    # Zh = 0.5 * Vh  (from bd quadrants, per half)
                    nc.vector.tensor_scalar_mul(
                        yz[0:64, :, 64:128], vbd[0:64, :, 0:64], 0.5)
                    nc.vector.tensor_scalar_mul(
                        yz[64:128, :, 64:128], vbd[64:128, :, 64:128], 0.5)
                else:
                    zbd = next_bd()
                    mirror_to_bd(yz[:, :, 64:128], zbd, nc.scalar)
                    psb = pbpool.tile([128, G, 64], F32, tag="psb")
                    for g in range(G):
                        nc.tensor.matmul(
                            psb[:, g, :], zbd[:, g, :], yz[:, g, 0:64],
                            start=True, stop=True)                 # M = Zh@Y
                    vbd = next_bd()
                    nc.vector.scalar_tensor_tensor(
                        vbd[0:64, :, 0:64], psb[0:64], 2.0 * b,
                        aslice[0:64], op0=ALU.mult, op1=ALU.add)
                    nc.vector.scalar_tensor_tensor(
                        vbd[64:128, :, 64:128], psb[64:128], 2.0 * b,
                        aslice[64:128], op0=ALU.mult, op1=ALU.add)
                    psa = papool.tile([128, G, 128], F32, tag="psa")
                    for g in range(G):
                        nc.tensor.matmul(
                            psa[:, g, :], vbd[:, g, :], yz[:, g, :],
                            start=True, stop=True)                 # [Y'|Zh']
                    yz2 = eypool.tile([128, G, 128], F32, tag="ey")
                    nc.scalar.copy(yz2[:], psa[:])
                    yz = yz2
                it += 1

            # ---- W = 0.5*Y - Zh ; U = W@W ; odd poly ----
            wst = tpool.tile([128, G, 64], F32, tag="wst")
            nc.vector.scalar_tensor_tensor(
                wst[:], yz[:, :, 0:64], 0.5, yz[:, :, 64:128],
                op0=ALU.mult, op1=ALU.subtract)
            wbd = next_bd()
            mirror_to_bd(wst[:], wbd, nc.scalar)
            psb = pbpool.tile([128, G, 64], F32, tag="psb")
            for g in range(G):
                nc.tensor.matmul(psb[:, g, :], wbd[:, g, :], wst[:, g, :],
                                 start=True, stop=True)            # U = W@W
            usb = tpool.tile([128, G, 64], F32, tag="usb")
            nc.scalar.copy(usb[:], psb[:])
            ubd = next_bd()
            mirror_to_bd(usb[:], ubd, nc.scalar)
            tacc = tpool.tile([128, G, 64], F32, tag="tacc")
            nc.vector.scalar_tensor_tensor(
                tacc[:], usb[:], POLY[4], kslice(1 + N_EITER + N_ZITER),
                op0=ALU.mult, op1=ALU.add)                         # p4*U + p3*I
            for j in (2, 1, 0):
                psb = pbpool.tile([128, G, 64], F32, tag="psb")
                for g in range(G):
                    nc.tensor.matmul(psb[:, g, :], ubd[:, g, :], tacc[:, g, :],
                                     start=True, stop=True)        # U@T
                tacc2 = tpool.tile([128, G, 64], F32, tag="tacc")
                nc.vector.scalar_tensor_tensor(
                    tacc2[:], psb[:], 1.0, kslice(1 + N_EITER + N_ZITER + (3 - j)),
                    op0=ALU.mult, op1=ALU.add)
                tacc = tacc2
            psb = pbpool.tile([128, G, 64], F32, tag="psb")
            for g in range(G):
                nc.tensor.matmul(psb[:, g, :], wbd[:, g, :], tacc[:, g, :],
                                 start=True, stop=True)            # W @ P'(U)
            out_t = iopool.tile([128, G, 64], F16, tag="out")
            nc.vector.scalar_tensor_tensor(
                out_t[:], psb[:], 1.0, kslice(NCONST - 1),
                op0=ALU.mult, op1=ALU.add)                         # + ln(c) I
            nc.sync.dma_start(
                out=y_out[blk].transpose([1, 2, 0, 3]), in_=out_t[:])

        with tc.For_i(0, nb, 1) as i:
            body(i)

    _legalize_waits(nc)
    return nc


_STATE = {}


def _get_state():
    if _STATE:
        return _STATE
    import jax
    import jax.numpy as jnp

    nc = build_nc()
    b2j.install_neuronx_cc_hook()

    partition_name = (
        nc.partition_id_tensor.name if nc.partition_id_tensor else None
    )
    in_names, out_names, out_avals = [], [], []
    for alloc in nc.m.functions[0].allocations:
        if not isinstance(alloc, mybir.MemoryLocationSet):
            continue
        name = alloc.memorylocations[0].name
        if alloc.kind == "ExternalInput":
            if name != partition_name:
                in_names.append(name)
        elif alloc.kind == "ExternalOutput":
            shape = tuple(alloc.tensor_shape)
            dtype = mybir.dt.np(alloc.dtype)
            out_names.append(name)
            out_avals.append(jax.core.ShapedArray(shape, dtype))
    n_params = len(in_names)
    in_names.extend(out_names)
    if partition_name is not None:
        in_names.append(partition_name)

    dbg_zero = None
    if nc.dbg_addr is not None:
        assert not nc.dbg_callbacks
        dbg_zero = np.zeros((1, 2), np.uint32)

    def _body(*args):
        operands = list(args)
        if partition_name is not None:
            operands.append(b2j.partition_id_tensor())
        outs = b2j._bass_exec_p.bind(
            *operands,
            out_avals=tuple(out_avals),
            in_names=tuple(in_names),
            out_names=tuple(out_names),
            lowering_input_output_aliases=(),
            sim_require_finite=True,
            sim_require_nnan=True,
            nc=nc,
        )
        return tuple(outs)

    devices = jax.devices()[:N_CORES]
    mesh = b2j.Mesh(np.asarray(devices), ("core",))
    P = b2j.PartitionSpec
    n_ops = n_params + len(out_names) + (1 if dbg_zero is not None else 0)
    # dbg input (if any) sits between declared params and out dummies in
    # in_names order: actually dbg_addr is an ExternalInput so it is already
    # in in_names via the allocation walk unless it equals partition_name.
    in_specs = (P("core"),) * n_ops
    out_specs = (P("core"),) * len(out_names)
    fn = jax.jit(
        b2j.shard_map(
            _body, mesh=mesh, in_specs=in_specs, out_specs=out_specs,
            check_rep=False,
        ),
        keep_unused=True,
    )

    from jax.sharding import NamedSharding
    sh = NamedSharding(mesh, P("core"))
    kbank = _host_consts()
    konst_dev = jax.device_put(
        np.broadcast_to(kbank, (N_CORES, *kbank.shape)).reshape(
            N_CORES * 128, NCONST, 64
        ),
        sh,
    )
    y_dummy = jax.jit(
        lambda: jnp.zeros((N_CORES * NB, G, 2, 64, 64), jnp.float16),
        out_shardings=sh,
    )()
    y_dummy.block_until_ready()

    _STATE.update(
        fn=fn, konst_dev=konst_dev, y_dummy=y_dummy, dbg_zero=dbg_zero,
        mesh=mesh, sh=sh, n_params=n_params,
    )
    return _STATE


def kernel(x: np.ndarray) -> np.ndarray:
    assert x.shape == (B_TOTAL, N, N)
    st = _get_state()

    x = np.ascontiguousarray(x, dtype=np.float32)
    hi = x.astype(np.float16)
    r = np.subtract(x, hi, dtype=np.float32)
    np.multiply(r, RES_SCALE, out=r)
    np.rint(r, out=r)
    np.clip(r, -127.0, 127.0, out=r)
    lo = r.astype(np.int8)

    xh_g = hi.reshape(N_CORES * NB, G, 2, 64, 64)
    xl_g = lo.reshape(N_CORES * NB, G, 2, 64, 64)

    args = [xh_g, xl_g, st["konst_dev"], st["y_dummy"]]
    if st["dbg_zero"] is not None:
        args.append(
            np.broadcast_to(st["dbg_zero"], (N_CORES, 2)).copy()
        )
    (y_dev,) = st["fn"](*args)
    y = np.asarray(y_dev)
    return y.reshape(B_TOTAL, N, N).astype(np.float32)
